# revision 31
# baseline (speedup 1.0000x reference)
"""Trainium2 Bass kernel for nn_DocSelfAttention.

Reference computation (per batch b):
    diff[e,a,h]  = wa[a,h] - ww[e,h]
    h3[e,a,m]    = tanh(diff @ w1 + b1)
    scores[e,a]  = h3 @ w2 + b2
    attn         = softmax(scores, axis=a)        (b2 cancels)
    pooled[e,h]  = attn @ wa
    out[e,m]     = (pooled + ww) @ w3 + b3

Key factorization: diff @ w1 = (wa @ w1)[a] - (ww @ w1)[e], so the big
[E,A,H]x[H,M] einsum collapses to two small matmuls plus a broadcast
subtract.  The kernel is then ACT-bound on the E*A*M = 16.7M-element tanh
per core (1 elem/cycle/lane @ 1.2 GHz ~= 112us).

Sharding: data-parallel over batch, one batch element per core (B=8).

Per-core dataflow (partition dim first):
    uT[m,a]    = (wa @ w1 + b1)^T     bf16
    vT[m,e]    = (ww @ w1)^T          f32 (per-partition scalar source)
    s/h tiles  [128m, G*512a]         bf16: tensor_scalar sub, ACT tanh
    scoresT    psum [128 a_loc, (ac,e)] via per-column matmuls
               (lhsT = h-slice [128m,128a], rhs = w2 chunk [128m,1])
    pooledT    psum [128h, 128e] = sum_ac wa_chunk.T @ expT_chunk
               (unnormalized; softmax denominator folded in at the end:
                out = rden (*) (pooledT.T @ w3) + (ww @ w3 + b3))

Walrus on this stack accepts at most ONE sync wait per engine
instruction, so the kernel maintains each engine's vector clock
explicitly: tiny PE "absorber" matmuls consume DMA/memset completions
phase by phase, and tiny DVE memsets into the fresh s/h tile slots take
over the slot-WAR waits that would otherwise land as a second wait on
the subs/tanh instructions.
"""

import numpy as np
from contextlib import ExitStack

import bass_rust
import concourse.bass as bass
import concourse.mybir as mybir
import concourse.tile as tile
from concourse.bass_utils import run_bass_kernel_spmd

F32 = mybir.dt.float32
BF16 = mybir.dt.bfloat16
AF = mybir.ActivationFunctionType
ALU = mybir.AluOpType

B, A, E, H, M = 8, 512, 128, 512, 256
P = 128
HC, MC, AC = H // P, M // P, A // P  # 4, 2, 4
G = 16                               # e-group size for sub/tanh tiles
NG = E // G                          # 8 groups

N_CORES = 8


def _build_kernel(ng=NG):
    nc = bass.Bass("TRN2", num_devices=N_CORES)

    wa_d = nc.dram_tensor("wa", [A, H], F32, kind="ExternalInput").ap()
    ww_d = nc.dram_tensor("ww", [E, H], F32, kind="ExternalInput").ap()
    w1_d = nc.dram_tensor("w1", [H, M], F32, kind="ExternalInput").ap()
    b1_d = nc.dram_tensor("b1", [M], F32, kind="ExternalInput").ap()
    w2_d = nc.dram_tensor("w2", [M], F32, kind="ExternalInput").ap()
    w3_d = nc.dram_tensor("w3", [H, M], F32, kind="ExternalInput").ap()
    b3_d = nc.dram_tensor("b3", [M], F32, kind="ExternalInput").ap()
    out_d = nc.dram_tensor("out", [E, M], F32, kind="ExternalOutput").ap()

    ident_d = nc.inline_tensor(np.eye(P, dtype=np.float32), name="ident").ap()

    with tile.TileContext(nc) as tc:
        with ExitStack() as ctx:
            _body(ctx, tc, nc, wa_d, ww_d, w1_d, b1_d, w2_d, w3_d, b3_d,
                  out_d, ident_d, ng)
    return nc


def _body(ctx, tc, nc, wa_d, ww_d, w1_d, b1_d, w2_d, w3_d, b3_d, out_d,
          ident_d, ng=NG):
    const = ctx.enter_context(tc.tile_pool(name="const", bufs=1))
    s_pool = ctx.enter_context(tc.tile_pool(name="s_pool", bufs=3))
    h_pool = ctx.enter_context(tc.tile_pool(name="h_pool", bufs=3))
    scr_pool = ctx.enter_context(tc.tile_pool(name="scr_pool", bufs=40))

    # ---- input DMAs ---------------------------------------------------
    hw_loads = []
    sw_loads = []

    ident = const.tile([P, P], F32)
    ident_load = nc.sync.dma_start(out=ident, in_=ident_d)

    act_warm = const.tile([1, 1], F32)
    warm = nc.scalar.activation(out=act_warm, in_=ident[0:1, 0:1],
                                func=AF.Tanh)

    wa_sb = []
    for ac in range(AC):
        t = const.tile([P, H], F32, name=f"wa_sb{ac}")
        hw_loads.append(nc.sync.dma_start(
            out=t, in_=wa_d[ac * P:(ac + 1) * P, :]))
        wa_sb.append(t)

    ww_sb = const.tile([P, H], F32)
    hw_loads.append(nc.sync.dma_start(out=ww_sb, in_=ww_d))
    phaseA = [ident_load] + list(hw_loads)

    w1_sb = []   # f32 (for vT)
    w1_bf = []   # bf16 (for uT)
    w3_sb = []
    for hc in range(HC):
        t1 = const.tile([P, M], F32, name=f"w1_sb{hc}")
        hw_loads.append(nc.sync.dma_start(
            out=t1, in_=w1_d[hc * P:(hc + 1) * P, :]))
        w1_sb.append(t1)
        t1b = const.tile([P, M], BF16, name=f"w1_bf{hc}")
        sw_loads.append(nc.gpsimd.dma_start(
            out=t1b, in_=w1_d[hc * P:(hc + 1) * P, :]))
        w1_bf.append(t1b)
        t3 = const.tile([P, M], F32, name=f"w3_sb{hc}")
        hw_loads.append(nc.sync.dma_start(
            out=t3, in_=w3_d[hc * P:(hc + 1) * P, :]))
        w3_sb.append(t3)

    b1_bf = const.tile([1, M], BF16)
    sw_loads.append(nc.gpsimd.dma_start(
        out=b1_bf, in_=b1_d.rearrange("(o m) -> o m", o=1)))
    b3_sb = const.tile([1, M], F32)
    hw_loads.append(nc.sync.dma_start(
        out=b3_sb, in_=b3_d.rearrange("(o m) -> o m", o=1)))

    # w2 as [128, 2] bf16 (cast during SWDGE DMA); column c = chunk c
    w2_sb = const.tile([P, MC], BF16)
    w2_load = nc.gpsimd.dma_start(
        out=w2_sb, in_=w2_d.rearrange("(c p) -> p c", p=P))
    sw_loads.append(w2_load)

    ones_bf = const.tile([1, A], BF16)
    m1 = nc.gpsimd.memset(ones_bf, 1.0)
    ones_f = const.tile([1, A], F32)
    m2 = nc.gpsimd.memset(ones_f, 1.0)
    ones_cb = const.tile([P, 1], BF16)
    pool_last = nc.gpsimd.memset(ones_cb, 1.0)

    phaseB = list(hw_loads[5:]) + sw_loads + [m1, m2, pool_last]

    # ---- psum phase A -------------------------------------------------
    wwT_sb = []
    waT_bf = [const.tile([P, A], BF16, name=f"waT_bf{hc}")
              for hc in range(HC)]
    wa_bf = [const.tile([P, H], BF16, name=f"wa_bf{ac}")
             for ac in range(AC)]
    uT_sb = []
    vT_sb = []

    with tc.tile_pool(name="ps_a", bufs=1, space="PSUM") as ps_a:
        prime_ps = ps_a.tile([1, 1], F32, tag="prime", name="prime_ps")

        def absorb(dep, reason):
            mm = nc.tensor.matmul(
                prime_ps, ident[0:1, 0:1], ident[0:1, 0:1],
                start=True, stop=True)
            bass_rust.add_dep_helper(
                mm.ins, dep.ins, sync=True, reason=reason)
            return mm

        last_abs = None
        for k, ld in enumerate(phaseA):
            last_abs = absorb(ld, f"pe-primeA-{k}")

        def ordered(ins):
            bass_rust.add_dep_helper(
                ins.ins, last_abs.ins, sync=False, reason="pe-order")
            return ins

        # ---- waT (cast to bf16) / wwT (f32) via PE transpose ----------
        startup_ops = []
        for hc in range(HC):
            for ac in range(AC):
                ptile = ps_a.tile([P, P], F32, tag="tww", bufs=4,
                                  name="pt_wa")
                ordered(nc.tensor.transpose(
                    out=ptile, in_=wa_sb[ac][:, hc * P:(hc + 1) * P],
                    identity=ident))
                startup_ops.append(nc.vector.tensor_copy(
                    out=waT_bf[hc][:, ac * P:(ac + 1) * P], in_=ptile))
        for hc in range(HC):
            ptile = ps_a.tile([P, P], F32, tag="tww", bufs=4, name="pt_ww")
            ordered(nc.tensor.transpose(
                out=ptile, in_=ww_sb[:, hc * P:(hc + 1) * P],
                identity=ident))
            t = const.tile([P, P], F32, name=f"wwT_sb{hc}")
            startup_ops.append(nc.vector.tensor_copy(out=t, in_=ptile))
            wwT_sb.append(t)

        # bf16 copies of wa (pooledT stationary later)
        for ac in range(AC):
            startup_ops.append(
                nc.vector.tensor_copy(out=wa_bf[ac], in_=wa_sb[ac]))

        # phase-B absorbers (w1/w3/b1/b3/w2/ones ready before u/v)
        for k, ld in enumerate(phaseB):
            last_abs = absorb(ld, f"pe-primeB-{k}")

        # ---- uT = (wa @ w1 + b1)^T (bf16), vT = (ww @ w1)^T (f32) -----
        for mc in range(MC):
            pu = ps_a.tile([P, A], F32, tag="mm512", bufs=2, name="pu")
            for hc in range(HC):
                ordered(nc.tensor.matmul(
                    pu, w1_bf[hc][:, mc * P:(mc + 1) * P], waT_bf[hc],
                    start=(hc == 0), stop=False))
            ordered(nc.tensor.matmul(
                pu, b1_bf[0:1, mc * P:(mc + 1) * P], ones_bf,
                start=False, stop=True))
            ut = const.tile([P, A], BF16, name=f"uT_sb{mc}")
            startup_ops.append(nc.vector.tensor_copy(out=ut, in_=pu))
            uT_sb.append(ut)

            pv = ps_a.tile([P, P], F32, tag="v128", bufs=1, name="pv")
            for hc in range(HC):
                startup_ops.append(ordered(nc.tensor.matmul(
                    pv, w1_sb[hc][:, mc * P:(mc + 1) * P], wwT_sb[hc],
                    start=(hc == 0), stop=(hc == HC - 1))))
            vt = const.tile([P, P], F32, name=f"vT_sb{mc}")
            startup_ops.append(nc.vector.tensor_copy(out=vt, in_=pv))
            vT_sb.append(vt)

        # absorb all startup copies/matmuls so main-loop PE instructions
        # carry at most one fresh wait
        for k, op in enumerate(startup_ops):
            last_abs = absorb(op, f"pe-primeC-{k}")

    # ---- main loop ----------------------------------------------------
    ps_b = ctx.enter_context(tc.tile_pool(name="ps_b", bufs=1, space="PSUM"))

    # scoresT psum column (ac*128 + e) holds scores[e, ac*128 + p].
    # Separate banks per m-chunk; every matmul is its own accumulation
    # group (start=stop=True) so column order is unconstrained.
    psum_s = [ps_b.tile([P, A], F32, name=f"psum_s{mc}", tag=f"sc{mc}")
              for mc in range(MC)]

    def dve_absorb(dep, reason):
        t = scr_pool.tile([1, 1], F32, tag="dscr", name="dscr")
        ab = nc.vector.memset(t, 0.0)
        bass_rust.add_dep_helper(ab.ins, dep.ins, sync=True, reason=reason)
        return ab

    def act_absorb(dep, reason):
        t = scr_pool.tile([1, 1], F32, tag="ascr", name="ascr")
        ab = nc.scalar.copy(out=t, in_=nc.const_aps.tensor(0.0, (1, 1), F32))
        bass_rust.add_dep_helper(ab.ins, dep.ins, sync=True, reason=reason)
        return ab

    # Per-iteration absorbers keep every DVE/ACT instruction at <=1 sync
    # wait: the s-slot WAR (a previous tanh) is absorbed by a tiny DVE
    # memset, the h-slot WAR (previous scores matmuls) and the sub->tanh
    # data wait by two tiny ACT copies (the tanh's waits then collapse to
    # one ACT-own wait).
    NBUF = 3
    tanh_ins = []
    mm_last = []
    it = 0
    for mc in range(MC):
        for g in range(ng):
            if it >= NBUF:
                dve_absorb(tanh_ins[it - NBUF], "dve-slot-abs")
            s_tile = s_pool.tile([P, G * A], BF16, tag="s", name="s_tile")
            for j in range(G):
                e = g * G + j
                sub = nc.vector.tensor_scalar(
                    out=s_tile[:, j * A:(j + 1) * A],
                    in0=uT_sb[mc],
                    scalar1=vT_sb[mc][:, e:e + 1],
                    scalar2=None,
                    op0=ALU.subtract)
            if it >= NBUF:
                act_absorb(mm_last[it - NBUF], "act-slot-abs")
            act_absorb(sub, "act-sub-abs")
            h_tile = h_pool.tile([P, G * A], BF16, tag="h", name="h_tile")
            tanh_ins.append(
                nc.scalar.activation(out=h_tile, in_=s_tile, func=AF.Tanh))
            for j in range(G):
                e = g * G + j
                for ac in range(AC):
                    col = ac * P + e
                    mm = nc.tensor.matmul(
                        psum_s[mc][:, col:col + 1],
                        h_tile[:, j * A + ac * P: j * A + (ac + 1) * P],
                        w2_sb[:, mc:mc + 1],
                        start=True, stop=True)
            mm_last.append(mm)
            it += 1

    # ---- softmax pieces -----------------------------------------------

    dve_absorb(mm_last[-1], "dve-tail-abs")
    scores_sb = const.tile([P, A], F32)
    nc.vector.tensor_copy(out=scores_sb, in_=psum_s[0])
    nc.vector.tensor_tensor(
        out=scores_sb, in0=scores_sb, in1=psum_s[1], op=ALU.add)
    expT_bf = const.tile([P, A], BF16)
    sc_exp = nc.scalar.activation(out=expT_bf, in_=scores_sb, func=AF.Exp)

    pden = ps_b.tile([P, 1], F32, tag="den")
    for ac in range(AC):
        nc.tensor.matmul(
            pden, expT_bf[:, ac * P:(ac + 1) * P], ones_cb,
            start=(ac == 0), stop=(ac == AC - 1))
    rden_sb = const.tile([P, 1], F32)
    nc.vector.reciprocal(out=rden_sb, in_=pden)

    # ---- pooledT [h, e] (unnormalized, bf16 inputs) -------------------
    poolT_sb = []
    for hc in range(HC):
        ppt = ps_b.tile([P, P], F32, tag="pT", bufs=2, name="ppt")
        for ac in range(AC):
            nc.tensor.matmul(
                ppt, wa_bf[ac][:, hc * P:(hc + 1) * P],
                expT_bf[:, ac * P:(ac + 1) * P],
                start=(ac == 0), stop=(ac == AC - 1))
        t = const.tile([P, P], F32, name=f"poolT_sb{hc}")
        nc.vector.tensor_copy(out=t, in_=ppt)
        poolT_sb.append(t)

    # ---- final: out = rden * (poolT.T @ w3) + (ww @ w3 + b3) ----------
    pq1 = ps_b.tile([P, M], F32, tag="q1")
    pq2 = ps_b.tile([P, M], F32, tag="q2")
    for hc in range(HC):
        q1_last = nc.tensor.matmul(pq1, poolT_sb[hc], w3_sb[hc],
                                   start=(hc == 0), stop=(hc == HC - 1))
        nc.tensor.matmul(pq2, wwT_sb[hc], w3_sb[hc],
                         start=(hc == 0), stop=False)
    q2_last = nc.tensor.matmul(pq2, ones_f[0:1, 0:P], b3_sb,
                               start=False, stop=True)

    dve_absorb(q1_last, "dve-q1-abs")
    t1_sb = const.tile([P, M], F32)
    nc.vector.tensor_scalar(
        out=t1_sb, in0=pq1, scalar1=rden_sb, scalar2=None, op0=ALU.mult)
    dve_absorb(q2_last, "dve-q2-abs")
    out_sb = const.tile([P, M], F32)
    out_w = nc.vector.tensor_tensor(out=out_sb, in0=t1_sb, in1=pq2,
                                    op=ALU.add)
    # Output via SWDGE: HWDGE DMAs always carry an own-lane FIFO wait, so
    # lane+data would exceed the 1-wait limit.  The SWDGE lane set has a
    # virgin lane here, leaving only the DVE data wait.
    out_dma = nc.gpsimd.dma_start(out=out_d, in_=out_sb)

    # SP nop joins: bring SP's vector clock up to date on every loose sem
    # end so the Tile kernel-tail drain needs no sync waits of its own.
    tail_deps = [out_dma, q2_last, q1_last, mm_last[-1], out_w, sc_exp,
                 pool_last, warm, ident_load]
    tail_deps += hw_loads + sw_loads
    for k, dep in enumerate(tail_deps):
        nop = nc.sync.nop(nofuse=True)
        bass_rust.add_dep_helper(
            nop.ins, dep.ins, sync=True, reason=f"sp-tail-join-{k}")


_NC_CACHE = None


def _get_nc():
    global _NC_CACHE
    if _NC_CACHE is None:
        _NC_CACHE = _build_kernel()
    return _NC_CACHE


def kernel(**inputs):
    wa = np.ascontiguousarray(np.asarray(inputs["word_all"], dtype=np.float32))
    ww = np.ascontiguousarray(
        np.asarray(inputs["word_weighted"], dtype=np.float32))
    w1 = np.ascontiguousarray(np.asarray(inputs["w1"], dtype=np.float32))
    b1 = np.ascontiguousarray(np.asarray(inputs["b1"], dtype=np.float32))
    w2 = np.ascontiguousarray(np.asarray(inputs["w2"], dtype=np.float32))
    w3 = np.ascontiguousarray(np.asarray(inputs["w3"], dtype=np.float32))
    b3 = np.ascontiguousarray(np.asarray(inputs["b3"], dtype=np.float32))
    # b2 is a pre-softmax additive constant: softmax(x + c) == softmax(x).

    nc = _get_nc()
    in_maps = [
        {
            "wa": np.ascontiguousarray(wa[b]),
            "ww": np.ascontiguousarray(ww[b]),
            "w1": w1,
            "b1": b1,
            "w2": w2,
            "w3": w3,
            "b3": b3,
        }
        for b in range(N_CORES)
    ]
    res = run_bass_kernel_spmd(nc, in_maps, core_ids=list(range(N_CORES)))
    return np.stack([res.results[b]["out"] for b in range(N_CORES)], axis=0)


# revision 32
# speedup vs baseline: 1.0453x; 1.0453x over previous
"""Trainium2 Bass kernel for nn_DocSelfAttention.

Reference computation (per batch b):
    diff[e,a,h]  = wa[a,h] - ww[e,h]
    h3[e,a,m]    = tanh(diff @ w1 + b1)
    scores[e,a]  = h3 @ w2 + b2
    attn         = softmax(scores, axis=a)        (b2 cancels)
    pooled[e,h]  = attn @ wa
    out[e,m]     = (pooled + ww) @ w3 + b3

Key factorization: diff @ w1 = (wa @ w1)[a] - (ww @ w1)[e], so the big
[E,A,H]x[H,M] einsum collapses to two small matmuls plus a broadcast
subtract.  The kernel is then ACT-bound on the E*A*M = 16.7M-element tanh
per core (1 elem/cycle/lane @ 1.2 GHz ~= 112us).

Sharding: data-parallel over batch, one batch element per core (B=8).

Per-core dataflow (partition dim first):
    uT[m,a]    = (wa @ w1 + b1)^T     bf16
    vT[m,e]    = (ww @ w1)^T          f32 (per-partition scalar source)
    s/h tiles  [128m, G*512a]         bf16: tensor_scalar sub, ACT tanh
    scoresT    psum [128 a_loc, (ac,e)] via per-column matmuls
               (lhsT = h-slice [128m,128a], rhs = w2 chunk [128m,1])
    pooledT    psum [128h, 128e] = sum_ac wa_chunk.T @ expT_chunk
               (unnormalized; softmax denominator folded in at the end:
                out = rden (*) (pooledT.T @ w3) + (ww @ w3 + b3))

Walrus on this stack accepts at most ONE sync wait per engine
instruction, so the kernel maintains each engine's vector clock
explicitly: tiny PE "absorber" matmuls consume DMA/memset completions
phase by phase, and tiny DVE memsets into the fresh s/h tile slots take
over the slot-WAR waits that would otherwise land as a second wait on
the subs/tanh instructions.
"""

import numpy as np
from contextlib import ExitStack

import bass_rust
import concourse.bass as bass
import concourse.mybir as mybir
import concourse.tile as tile
from concourse.bass_utils import run_bass_kernel_spmd

F32 = mybir.dt.float32
BF16 = mybir.dt.bfloat16
AF = mybir.ActivationFunctionType
ALU = mybir.AluOpType

B, A, E, H, M = 8, 512, 128, 512, 256
P = 128
HC, MC, AC = H // P, M // P, A // P  # 4, 2, 4
G = 16                               # e-group size for sub/tanh tiles
NG = E // G                          # 8 groups

N_CORES = 8


def _build_kernel(ng=NG):
    nc = bass.Bass("TRN2", num_devices=N_CORES)

    wa_d = nc.dram_tensor("wa", [A, H], F32, kind="ExternalInput").ap()
    ww_d = nc.dram_tensor("ww", [E, H], F32, kind="ExternalInput").ap()
    w1_d = nc.dram_tensor("w1", [H, M], F32, kind="ExternalInput").ap()
    b1_d = nc.dram_tensor("b1", [M], F32, kind="ExternalInput").ap()
    w2_d = nc.dram_tensor("w2", [M], F32, kind="ExternalInput").ap()
    w3_d = nc.dram_tensor("w3", [H, M], F32, kind="ExternalInput").ap()
    b3_d = nc.dram_tensor("b3", [M], F32, kind="ExternalInput").ap()
    out_d = nc.dram_tensor("out", [E, M], F32, kind="ExternalOutput").ap()

    ident_d = nc.inline_tensor(np.eye(P, dtype=np.float32), name="ident").ap()

    with tile.TileContext(nc) as tc:
        with ExitStack() as ctx:
            _body(ctx, tc, nc, wa_d, ww_d, w1_d, b1_d, w2_d, w3_d, b3_d,
                  out_d, ident_d, ng)
    return nc


def _body(ctx, tc, nc, wa_d, ww_d, w1_d, b1_d, w2_d, w3_d, b3_d, out_d,
          ident_d, ng=NG):
    const = ctx.enter_context(tc.tile_pool(name="const", bufs=1))
    s_pool = ctx.enter_context(tc.tile_pool(name="s_pool", bufs=3))
    h_pool = ctx.enter_context(tc.tile_pool(name="h_pool", bufs=3))
    scr_pool = ctx.enter_context(tc.tile_pool(name="scr_pool", bufs=40))

    # ---- input DMAs ---------------------------------------------------
    hw_loads = []
    sw_loads = []

    ident = const.tile([P, P], F32)
    ident_load = nc.sync.dma_start(out=ident, in_=ident_d)

    act_warm = const.tile([1, 1], F32)
    warm = nc.scalar.activation(out=act_warm, in_=ident[0:1, 0:1],
                                func=AF.Tanh)

    wa_all = const.tile([P, AC, H], F32)
    hw_loads.append(nc.sync.dma_start(
        out=wa_all, in_=wa_d.rearrange("(c p) h -> p c h", p=P)))
    wa_sb = [wa_all[:, ac, :] for ac in range(AC)]

    ww_sb = const.tile([P, H], F32)
    hw_loads.append(nc.sync.dma_start(out=ww_sb, in_=ww_d))
    phaseA = [ident_load] + list(hw_loads)

    w1_all = const.tile([P, HC, M], F32)
    hw_loads.append(nc.sync.dma_start(
        out=w1_all, in_=w1_d.rearrange("(c p) m -> p c m", p=P)))
    w1_sb = [w1_all[:, hc, :] for hc in range(HC)]
    w1_ball = const.tile([P, HC, M], BF16)
    sw_loads.append(nc.gpsimd.dma_start(
        out=w1_ball, in_=w1_d.rearrange("(c p) m -> p c m", p=P)))
    w1_bf = [w1_ball[:, hc, :] for hc in range(HC)]
    w3_all = const.tile([P, HC, M], F32)
    hw_loads.append(nc.sync.dma_start(
        out=w3_all, in_=w3_d.rearrange("(c p) m -> p c m", p=P)))
    w3_sb = [w3_all[:, hc, :] for hc in range(HC)]

    b1_bf = const.tile([1, M], BF16)
    sw_loads.append(nc.gpsimd.dma_start(
        out=b1_bf, in_=b1_d.rearrange("(o m) -> o m", o=1)))
    b3_sb = const.tile([1, M], F32)
    hw_loads.append(nc.sync.dma_start(
        out=b3_sb, in_=b3_d.rearrange("(o m) -> o m", o=1)))

    # w2 as [128, 2] bf16 (cast during SWDGE DMA); column c = chunk c
    w2_sb = const.tile([P, MC], BF16)
    w2_load = nc.gpsimd.dma_start(
        out=w2_sb, in_=w2_d.rearrange("(c p) -> p c", p=P))
    sw_loads.append(w2_load)

    ones_bf = const.tile([1, A], BF16)
    m1 = nc.gpsimd.memset(ones_bf, 1.0)
    ones_f = const.tile([1, A], F32)
    m2 = nc.gpsimd.memset(ones_f, 1.0)
    ones_cb = const.tile([P, 1], BF16)
    pool_last = nc.gpsimd.memset(ones_cb, 1.0)

    phaseB = list(hw_loads[2:]) + sw_loads + [m1, m2, pool_last]

    # ---- psum phase A -------------------------------------------------
    wwT_sb = []
    waT_bf = [const.tile([P, A], BF16, name=f"waT_bf{hc}")
              for hc in range(HC)]
    wa_bf = [const.tile([P, H], BF16, name=f"wa_bf{ac}")
             for ac in range(AC)]
    uT_sb = []
    vT_sb = []

    with tc.tile_pool(name="ps_a", bufs=1, space="PSUM") as ps_a:
        prime_ps = ps_a.tile([1, 1], F32, tag="prime", name="prime_ps")

        def absorb(dep, reason):
            mm = nc.tensor.matmul(
                prime_ps, ident[0:1, 0:1], ident[0:1, 0:1],
                start=True, stop=True)
            bass_rust.add_dep_helper(
                mm.ins, dep.ins, sync=True, reason=reason)
            return mm

        last_abs = None
        for k, ld in enumerate(phaseA):
            last_abs = absorb(ld, f"pe-primeA-{k}")

        def ordered(ins):
            bass_rust.add_dep_helper(
                ins.ins, last_abs.ins, sync=False, reason="pe-order")
            return ins

        # ---- waT (cast to bf16) / wwT (f32) via PE transpose ----------
        startup_ops = []
        last_T = None
        for hc in range(HC):
            for ac in range(AC):
                ptile = ps_a.tile([P, P], F32, tag="tww", bufs=4,
                                  name="pt_wa")
                last_T = ordered(nc.tensor.transpose(
                    out=ptile, in_=wa_sb[ac][:, hc * P:(hc + 1) * P],
                    identity=ident))
                startup_ops.append(nc.vector.tensor_copy(
                    out=waT_bf[hc][:, ac * P:(ac + 1) * P], in_=ptile))
        for hc in range(HC):
            ptile = ps_a.tile([P, P], F32, tag="tww", bufs=4, name="pt_ww")
            last_T = ordered(nc.tensor.transpose(
                out=ptile, in_=ww_sb[:, hc * P:(hc + 1) * P],
                identity=ident))
            t = const.tile([P, P], F32, name=f"wwT_sb{hc}")
            startup_ops.append(nc.vector.tensor_copy(out=t, in_=ptile))
            wwT_sb.append(t)

        # bf16 copies of wa (pooledT stationary later)
        for ac in range(AC):
            startup_ops.append(
                nc.vector.tensor_copy(out=wa_bf[ac], in_=wa_sb[ac]))

        # phase-B absorbers (w1/w3/b1/b3/w2/ones ready before u/v);
        # ordered AFTER the transposes so they don't stall them on the
        # PE FIFO while the weight DMAs are still in flight
        for k, ld in enumerate(phaseB):
            last_abs = absorb(ld, f"pe-primeB-{k}")
            bass_rust.add_dep_helper(
                last_abs.ins, last_T.ins, sync=False, reason="pe-orderB")

        # ---- uT = (wa @ w1 + b1)^T (bf16), vT = (ww @ w1)^T (f32) -----
        for mc in range(MC):
            pu = ps_a.tile([P, A], F32, tag="mm512", bufs=2, name="pu")
            for hc in range(HC):
                ordered(nc.tensor.matmul(
                    pu, w1_bf[hc][:, mc * P:(mc + 1) * P], waT_bf[hc],
                    start=(hc == 0), stop=False))
            ordered(nc.tensor.matmul(
                pu, b1_bf[0:1, mc * P:(mc + 1) * P], ones_bf,
                start=False, stop=True))
            ut = const.tile([P, A], BF16, name=f"uT_sb{mc}")
            startup_ops.append(nc.vector.tensor_copy(out=ut, in_=pu))
            uT_sb.append(ut)

            pv = ps_a.tile([P, P], F32, tag="v128", bufs=1, name="pv")
            for hc in range(HC):
                startup_ops.append(ordered(nc.tensor.matmul(
                    pv, w1_sb[hc][:, mc * P:(mc + 1) * P], wwT_sb[hc],
                    start=(hc == 0), stop=(hc == HC - 1))))
            vt = const.tile([P, P], F32, name=f"vT_sb{mc}")
            startup_ops.append(nc.vector.tensor_copy(out=vt, in_=pv))
            vT_sb.append(vt)

        # absorb all startup copies/matmuls so main-loop PE instructions
        # carry at most one fresh wait
        for k, op in enumerate(startup_ops):
            last_abs = absorb(op, f"pe-primeC-{k}")

    # ---- main loop ----------------------------------------------------
    ps_b = ctx.enter_context(tc.tile_pool(name="ps_b", bufs=1, space="PSUM"))

    # scoresT psum column (ac*128 + e) holds scores[e, ac*128 + p].
    # Separate banks per m-chunk; every matmul is its own accumulation
    # group (start=stop=True) so column order is unconstrained.
    psum_s = [ps_b.tile([P, A], F32, name=f"psum_s{mc}", tag=f"sc{mc}")
              for mc in range(MC)]

    def dve_absorb(dep, reason):
        t = scr_pool.tile([1, 1], F32, tag="dscr", name="dscr")
        ab = nc.vector.memset(t, 0.0)
        bass_rust.add_dep_helper(ab.ins, dep.ins, sync=True, reason=reason)
        return ab

    def act_absorb(dep, reason):
        t = scr_pool.tile([1, 1], F32, tag="ascr", name="ascr")
        ab = nc.scalar.copy(out=t, in_=nc.const_aps.tensor(0.0, (1, 1), F32))
        bass_rust.add_dep_helper(ab.ins, dep.ins, sync=True, reason=reason)
        return ab

    # Per-iteration absorbers keep every DVE/ACT instruction at <=1 sync
    # wait: the s-slot WAR (a previous tanh) is absorbed by a tiny DVE
    # memset, the h-slot WAR (previous scores matmuls) and the sub->tanh
    # data wait by two tiny ACT copies (the tanh's waits then collapse to
    # one ACT-own wait).
    NBUF = 3
    # Small leading groups shorten the path to the first tanh (the first
    # tanh must wait for its whole group's subs).
    groups0 = [4, 4, 8] + [G] * (ng - 1)
    assert sum(groups0) == E
    group_plan = [groups0] + [[G] * ng for _ in range(MC - 1)]
    tanh_ins = []
    mm_last = []
    it = 0
    for mc in range(MC):
        e0 = 0
        for gsz in group_plan[mc]:
            if it >= NBUF:
                dve_absorb(tanh_ins[it - NBUF], "dve-slot-abs")
            s_tile = s_pool.tile([P, gsz * A], BF16, tag="s", name="s_tile")
            for j in range(gsz):
                e = e0 + j
                sub = nc.vector.tensor_scalar(
                    out=s_tile[:, j * A:(j + 1) * A],
                    in0=uT_sb[mc],
                    scalar1=vT_sb[mc][:, e:e + 1],
                    scalar2=None,
                    op0=ALU.subtract)
            if it >= NBUF:
                act_absorb(mm_last[it - NBUF], "act-slot-abs")
            act_absorb(sub, "act-sub-abs")
            h_tile = h_pool.tile([P, gsz * A], BF16, tag="h", name="h_tile")
            tanh_ins.append(
                nc.scalar.activation(out=h_tile, in_=s_tile, func=AF.Tanh))
            for j in range(gsz):
                e = e0 + j
                for ac in range(AC):
                    col = ac * P + e
                    mm = nc.tensor.matmul(
                        psum_s[mc][:, col:col + 1],
                        h_tile[:, j * A + ac * P: j * A + (ac + 1) * P],
                        w2_sb[:, mc:mc + 1],
                        start=True, stop=True)
            mm_last.append(mm)
            e0 += gsz
            it += 1

    # ---- softmax pieces -----------------------------------------------

    dve_absorb(mm_last[-1], "dve-tail-abs")
    scores_sb = const.tile([P, A], F32)
    nc.vector.tensor_copy(out=scores_sb, in_=psum_s[0])
    nc.vector.tensor_tensor(
        out=scores_sb, in0=scores_sb, in1=psum_s[1], op=ALU.add)
    expT_bf = const.tile([P, A], BF16)
    sc_exp = nc.scalar.activation(out=expT_bf, in_=scores_sb, func=AF.Exp)

    pden = ps_b.tile([P, 1], F32, tag="den")
    for ac in range(AC):
        nc.tensor.matmul(
            pden, expT_bf[:, ac * P:(ac + 1) * P], ones_cb,
            start=(ac == 0), stop=(ac == AC - 1))
    rden_sb = const.tile([P, 1], F32)
    nc.vector.reciprocal(out=rden_sb, in_=pden)

    # ---- pooledT [h, e] (unnormalized, bf16 inputs) -------------------
    poolT_sb = []
    for hc in range(HC):
        ppt = ps_b.tile([P, P], F32, tag="pT", bufs=2, name="ppt")
        for ac in range(AC):
            nc.tensor.matmul(
                ppt, wa_bf[ac][:, hc * P:(hc + 1) * P],
                expT_bf[:, ac * P:(ac + 1) * P],
                start=(ac == 0), stop=(ac == AC - 1))
        t = const.tile([P, P], F32, name=f"poolT_sb{hc}")
        nc.vector.tensor_copy(out=t, in_=ppt)
        poolT_sb.append(t)

    # ---- final: out = rden * (poolT.T @ w3) + (ww @ w3 + b3) ----------
    pq1 = ps_b.tile([P, M], F32, tag="q1")
    pq2 = ps_b.tile([P, M], F32, tag="q2")
    for hc in range(HC):
        q1_last = nc.tensor.matmul(pq1, poolT_sb[hc], w3_sb[hc],
                                   start=(hc == 0), stop=(hc == HC - 1))
        nc.tensor.matmul(pq2, wwT_sb[hc], w3_sb[hc],
                         start=(hc == 0), stop=False)
    q2_last = nc.tensor.matmul(pq2, ones_f[0:1, 0:P], b3_sb,
                               start=False, stop=True)

    dve_absorb(q1_last, "dve-q1-abs")
    t1_sb = const.tile([P, M], F32)
    nc.vector.tensor_scalar(
        out=t1_sb, in0=pq1, scalar1=rden_sb, scalar2=None, op0=ALU.mult)
    dve_absorb(q2_last, "dve-q2-abs")
    out_sb = const.tile([P, M], F32)
    out_w = nc.vector.tensor_tensor(out=out_sb, in0=t1_sb, in1=pq2,
                                    op=ALU.add)
    # Output via SWDGE: HWDGE DMAs always carry an own-lane FIFO wait, so
    # lane+data would exceed the 1-wait limit.  The SWDGE lane set has a
    # virgin lane here, leaving only the DVE data wait.
    out_dma = nc.gpsimd.dma_start(out=out_d, in_=out_sb)

    # SP nop joins: bring SP's vector clock up to date on every loose sem
    # end so the Tile kernel-tail drain needs no sync waits of its own.
    tail_deps = [out_dma, q2_last, q1_last, mm_last[-1], out_w, sc_exp,
                 pool_last, warm, ident_load]
    tail_deps += hw_loads + sw_loads
    for k, dep in enumerate(tail_deps):
        nop = nc.sync.nop(nofuse=True)
        bass_rust.add_dep_helper(
            nop.ins, dep.ins, sync=True, reason=f"sp-tail-join-{k}")


_NC_CACHE = None


def _get_nc():
    global _NC_CACHE
    if _NC_CACHE is None:
        _NC_CACHE = _build_kernel()
    return _NC_CACHE


def kernel(**inputs):
    wa = np.ascontiguousarray(np.asarray(inputs["word_all"], dtype=np.float32))
    ww = np.ascontiguousarray(
        np.asarray(inputs["word_weighted"], dtype=np.float32))
    w1 = np.ascontiguousarray(np.asarray(inputs["w1"], dtype=np.float32))
    b1 = np.ascontiguousarray(np.asarray(inputs["b1"], dtype=np.float32))
    w2 = np.ascontiguousarray(np.asarray(inputs["w2"], dtype=np.float32))
    w3 = np.ascontiguousarray(np.asarray(inputs["w3"], dtype=np.float32))
    b3 = np.ascontiguousarray(np.asarray(inputs["b3"], dtype=np.float32))
    # b2 is a pre-softmax additive constant: softmax(x + c) == softmax(x).

    nc = _get_nc()
    in_maps = [
        {
            "wa": np.ascontiguousarray(wa[b]),
            "ww": np.ascontiguousarray(ww[b]),
            "w1": w1,
            "b1": b1,
            "w2": w2,
            "w3": w3,
            "b3": b3,
        }
        for b in range(N_CORES)
    ]
    res = run_bass_kernel_spmd(nc, in_maps, core_ids=list(range(N_CORES)))
    return np.stack([res.results[b]["out"] for b in range(N_CORES)], axis=0)


# revision 33
# speedup vs baseline: 4378.9649x; 4189.2185x over previous
"""Trainium2 Bass kernel for nn_DocSelfAttention.

Reference computation (per batch b):
    diff[e,a,h]  = wa[a,h] - ww[e,h]
    h3[e,a,m]    = tanh(diff @ w1 + b1)
    scores[e,a]  = h3 @ w2 + b2
    attn         = softmax(scores, axis=a)        (b2 cancels)
    pooled[e,h]  = attn @ wa
    out[e,m]     = (pooled + ww) @ w3 + b3

Key factorization: diff @ w1 = (wa @ w1)[a] - (ww @ w1)[e], so the big
[E,A,H]x[H,M] einsum collapses to two small matmuls plus a broadcast
subtract.  The kernel is then ACT-bound on the E*A*M = 16.7M-element tanh
per core (1 elem/cycle/lane @ 1.2 GHz ~= 112us).

Sharding: data-parallel over batch, one batch element per core (B=8).

Per-core dataflow (partition dim first):
    uT[m,a]    = (wa @ w1 + b1)^T     bf16
    vT[m,e]    = (ww @ w1)^T          f32 (per-partition scalar source)
    s/h tiles  [128m, G*512a]         bf16: tensor_scalar sub, ACT tanh
    scoresT    psum [128 a_loc, (ac,e)] via per-column matmuls
               (lhsT = h-slice [128m,128a], rhs = w2 chunk [128m,1])
    pooledT    psum [128h, 128e] = sum_ac wa_chunk.T @ expT_chunk
               (unnormalized; softmax denominator folded in at the end:
                out = rden (*) (pooledT.T @ w3) + (ww @ w3 + b3))

Walrus on this stack accepts at most ONE sync wait per engine
instruction, so the kernel maintains each engine's vector clock
explicitly: tiny PE "absorber" matmuls consume DMA/memset completions
phase by phase, and tiny DVE memsets into the fresh s/h tile slots take
over the slot-WAR waits that would otherwise land as a second wait on
the subs/tanh instructions.
"""

import numpy as np
from contextlib import ExitStack

import bass_rust
import concourse.bass as bass
import concourse.mybir as mybir
import concourse.tile as tile
from concourse.bass_utils import run_bass_kernel_spmd

F32 = mybir.dt.float32
BF16 = mybir.dt.bfloat16
AF = mybir.ActivationFunctionType
ALU = mybir.AluOpType

B, A, E, H, M = 8, 512, 128, 512, 256
P = 128
HC, MC, AC = H // P, M // P, A // P  # 4, 2, 4
G = 16                               # e-group size for sub/tanh tiles
NG = E // G                          # 8 groups

N_CORES = 8


def _build_kernel(ng=NG):
    nc = bass.Bass("TRN2", num_devices=N_CORES)

    wa_d = nc.dram_tensor("wa", [A, H], F32, kind="ExternalInput").ap()
    ww_d = nc.dram_tensor("ww", [E, H], F32, kind="ExternalInput").ap()
    w1_d = nc.dram_tensor("w1", [H, M], F32, kind="ExternalInput").ap()
    b1_d = nc.dram_tensor("b1", [M], F32, kind="ExternalInput").ap()
    w2_d = nc.dram_tensor("w2", [M], F32, kind="ExternalInput").ap()
    w3_d = nc.dram_tensor("w3", [H, M], F32, kind="ExternalInput").ap()
    b3_d = nc.dram_tensor("b3", [M], F32, kind="ExternalInput").ap()
    out_d = nc.dram_tensor("out", [E, M], F32, kind="ExternalOutput").ap()

    ident_d = nc.inline_tensor(np.eye(P, dtype=np.float32), name="ident").ap()

    with tile.TileContext(nc) as tc:
        with ExitStack() as ctx:
            _body(ctx, tc, nc, wa_d, ww_d, w1_d, b1_d, w2_d, w3_d, b3_d,
                  out_d, ident_d, ng)
    return nc


def _body(ctx, tc, nc, wa_d, ww_d, w1_d, b1_d, w2_d, w3_d, b3_d, out_d,
          ident_d, ng=NG):
    const = ctx.enter_context(tc.tile_pool(name="const", bufs=1))
    s_pool = ctx.enter_context(tc.tile_pool(name="s_pool", bufs=3))
    h_pool = ctx.enter_context(tc.tile_pool(name="h_pool", bufs=3))
    scr_pool = ctx.enter_context(tc.tile_pool(name="scr_pool", bufs=40))

    # ---- input DMAs ---------------------------------------------------
    hw_loads = []
    sw_loads = []

    ident = const.tile([P, P], F32)
    ident_load = nc.sync.dma_start(out=ident, in_=ident_d)

    act_warm = const.tile([1, 1], F32)
    warm = nc.scalar.activation(out=act_warm, in_=ident[0:1, 0:1],
                                func=AF.Tanh)

    wa_all = const.tile([P, AC, H], F32)
    hw_loads.append(nc.sync.dma_start(
        out=wa_all, in_=wa_d.rearrange("(c p) h -> p c h", p=P)))
    wa_sb = [wa_all[:, ac, :] for ac in range(AC)]

    ww_sb = const.tile([P, H], F32)
    hw_loads.append(nc.sync.dma_start(out=ww_sb, in_=ww_d))
    phaseA = [ident_load] + list(hw_loads)

    # keep the big wa DMA at the head of the SP DMA queue: everything on
    # the startup critical path waits for it
    wa_dma = hw_loads[0]
    bass_rust.add_dep_helper(
        hw_loads[1].ins, wa_dma.ins, sync=False, reason="dma-order-ww")

    w1_all = const.tile([P, HC, M], F32)
    _d = nc.sync.dma_start(
        out=w1_all, in_=w1_d.rearrange("(c p) m -> p c m", p=P))
    bass_rust.add_dep_helper(_d.ins, wa_dma.ins, sync=False,
                             reason="dma-order-w1")
    hw_loads.append(_d)
    w1_sb = [w1_all[:, hc, :] for hc in range(HC)]
    w1_ball = const.tile([P, HC, M], BF16)
    sw_loads.append(nc.gpsimd.dma_start(
        out=w1_ball, in_=w1_d.rearrange("(c p) m -> p c m", p=P)))
    w1_bf = [w1_ball[:, hc, :] for hc in range(HC)]
    w3_all = const.tile([P, HC, M], F32)
    _d = nc.sync.dma_start(
        out=w3_all, in_=w3_d.rearrange("(c p) m -> p c m", p=P))
    bass_rust.add_dep_helper(_d.ins, wa_dma.ins, sync=False,
                             reason="dma-order-w3")
    hw_loads.append(_d)
    w3_sb = [w3_all[:, hc, :] for hc in range(HC)]

    b1_bf = const.tile([1, M], BF16)
    sw_loads.append(nc.gpsimd.dma_start(
        out=b1_bf, in_=b1_d.rearrange("(o m) -> o m", o=1)))
    b3_sb = const.tile([1, M], F32)
    _d = nc.sync.dma_start(
        out=b3_sb, in_=b3_d.rearrange("(o m) -> o m", o=1))
    bass_rust.add_dep_helper(_d.ins, wa_dma.ins, sync=False,
                             reason="dma-order-b3")
    hw_loads.append(_d)

    # w2 as [128, 2] bf16 (cast during SWDGE DMA); column c = chunk c
    w2_sb = const.tile([P, MC], BF16)
    w2_load = nc.gpsimd.dma_start(
        out=w2_sb, in_=w2_d.rearrange("(c p) -> p c", p=P))
    sw_loads.append(w2_load)

    ones_bf = const.tile([1, A], BF16)
    m1 = nc.gpsimd.memset(ones_bf, 1.0)
    ones_f = const.tile([1, A], F32)
    m2 = nc.gpsimd.memset(ones_f, 1.0)
    ones_cb = const.tile([P, 1], BF16)
    pool_last = nc.gpsimd.memset(ones_cb, 1.0)

    phaseB = list(hw_loads[2:]) + sw_loads + [m1, m2, pool_last]

    # ---- psum phase A -------------------------------------------------
    wwT_sb = []
    waT_bf = [const.tile([P, A], BF16, name=f"waT_bf{hc}")
              for hc in range(HC)]
    wa_bf = [const.tile([P, H], BF16, name=f"wa_bf{ac}")
             for ac in range(AC)]
    uT_sb = []
    vT_sb = []

    with tc.tile_pool(name="ps_a", bufs=1, space="PSUM") as ps_a:
        prime_ps = ps_a.tile([1, 1], F32, tag="prime", name="prime_ps")

        def absorb(dep, reason):
            mm = nc.tensor.matmul(
                prime_ps, ident[0:1, 0:1], ident[0:1, 0:1],
                start=True, stop=True)
            bass_rust.add_dep_helper(
                mm.ins, dep.ins, sync=True, reason=reason)
            return mm

        last_abs = None
        for k, ld in enumerate(phaseA):
            last_abs = absorb(ld, f"pe-primeA-{k}")

        def ordered(ins):
            bass_rust.add_dep_helper(
                ins.ins, last_abs.ins, sync=False, reason="pe-order")
            return ins

        # ---- waT (cast to bf16) / wwT (f32) via PE transpose ----------
        startup_ops = []
        last_T = None
        for hc in range(HC):
            for ac in range(AC):
                ptile = ps_a.tile([P, P], F32, tag="tww", bufs=4,
                                  name="pt_wa")
                last_T = ordered(nc.tensor.transpose(
                    out=ptile, in_=wa_sb[ac][:, hc * P:(hc + 1) * P],
                    identity=ident))
                startup_ops.append(nc.vector.tensor_copy(
                    out=waT_bf[hc][:, ac * P:(ac + 1) * P], in_=ptile))
        for hc in range(HC):
            ptile = ps_a.tile([P, P], F32, tag="tww", bufs=4, name="pt_ww")
            last_T = ordered(nc.tensor.transpose(
                out=ptile, in_=ww_sb[:, hc * P:(hc + 1) * P],
                identity=ident))
            t = const.tile([P, P], F32, name=f"wwT_sb{hc}")
            startup_ops.append(nc.vector.tensor_copy(out=t, in_=ptile))
            wwT_sb.append(t)

        # bf16 copies of wa (pooledT stationary later)
        for ac in range(AC):
            startup_ops.append(
                nc.vector.tensor_copy(out=wa_bf[ac], in_=wa_sb[ac]))

        # phase-B absorbers (w1/w3/b1/b3/w2/ones ready before u/v);
        # ordered AFTER the transposes so they don't stall them on the
        # PE FIFO while the weight DMAs are still in flight
        for k, ld in enumerate(phaseB):
            last_abs = absorb(ld, f"pe-primeB-{k}")
            bass_rust.add_dep_helper(
                last_abs.ins, last_T.ins, sync=False, reason="pe-orderB")

        # ---- uT = (wa @ w1 + b1)^T (bf16), vT = (ww @ w1)^T (f32) -----
        for mc in range(MC):
            pu = ps_a.tile([P, A], F32, tag="mm512", bufs=2, name="pu")
            for hc in range(HC):
                ordered(nc.tensor.matmul(
                    pu, w1_bf[hc][:, mc * P:(mc + 1) * P], waT_bf[hc],
                    start=(hc == 0), stop=False))
            ordered(nc.tensor.matmul(
                pu, b1_bf[0:1, mc * P:(mc + 1) * P], ones_bf,
                start=False, stop=True))
            ut = const.tile([P, A], BF16, name=f"uT_sb{mc}")
            startup_ops.append(nc.vector.tensor_copy(out=ut, in_=pu))
            uT_sb.append(ut)

            pv = ps_a.tile([P, P], F32, tag="v128", bufs=1, name="pv")
            for hc in range(HC):
                startup_ops.append(ordered(nc.tensor.matmul(
                    pv, w1_sb[hc][:, mc * P:(mc + 1) * P], wwT_sb[hc],
                    start=(hc == 0), stop=(hc == HC - 1))))
            vt = const.tile([P, P], F32, name=f"vT_sb{mc}")
            startup_ops.append(nc.vector.tensor_copy(out=vt, in_=pv))
            vT_sb.append(vt)

        # absorb all startup copies/matmuls so main-loop PE instructions
        # carry at most one fresh wait
        for k, op in enumerate(startup_ops):
            last_abs = absorb(op, f"pe-primeC-{k}")

    # ---- main loop ----------------------------------------------------
    ps_b = ctx.enter_context(tc.tile_pool(name="ps_b", bufs=1, space="PSUM"))

    # scoresT psum column (ac*128 + e) holds scores[e, ac*128 + p].
    # Separate banks per m-chunk; every matmul is its own accumulation
    # group (start=stop=True) so column order is unconstrained.
    psum_s = [ps_b.tile([P, A], F32, name=f"psum_s{mc}", tag=f"sc{mc}")
              for mc in range(MC)]

    def dve_absorb(dep, reason):
        t = scr_pool.tile([1, 1], F32, tag="dscr", name="dscr")
        ab = nc.vector.memset(t, 0.0)
        bass_rust.add_dep_helper(ab.ins, dep.ins, sync=True, reason=reason)
        return ab

    def act_absorb(dep, reason):
        t = scr_pool.tile([1, 1], F32, tag="ascr", name="ascr")
        ab = nc.scalar.copy(out=t, in_=nc.const_aps.tensor(0.0, (1, 1), F32))
        bass_rust.add_dep_helper(ab.ins, dep.ins, sync=True, reason=reason)
        return ab

    # Per-iteration absorbers keep every DVE/ACT instruction at <=1 sync
    # wait: the s-slot WAR (a previous tanh) is absorbed by a tiny DVE
    # memset, the h-slot WAR (previous scores matmuls) and the sub->tanh
    # data wait by two tiny ACT copies (the tanh's waits then collapse to
    # one ACT-own wait).
    NBUF = 3
    # Small leading groups shorten the path to the first tanh (the first
    # tanh must wait for its whole group's subs).
    groups0 = [4, 4, 8] + [G] * (ng - 1)
    assert sum(groups0) == E
    group_plan = [groups0] + [[G] * ng for _ in range(MC - 1)]
    tanh_ins = []
    mm_last = []
    it = 0
    for mc in range(MC):
        e0 = 0
        for gsz in group_plan[mc]:
            if it >= NBUF:
                dve_absorb(tanh_ins[it - NBUF], "dve-slot-abs")
            s_tile = s_pool.tile([P, gsz * A], BF16, tag="s", name="s_tile")
            for j in range(gsz):
                e = e0 + j
                sub = nc.vector.tensor_scalar(
                    out=s_tile[:, j * A:(j + 1) * A],
                    in0=uT_sb[mc],
                    scalar1=vT_sb[mc][:, e:e + 1],
                    scalar2=None,
                    op0=ALU.subtract)
            if it >= NBUF:
                act_absorb(mm_last[it - NBUF], "act-slot-abs")
            act_absorb(sub, "act-sub-abs")
            h_tile = h_pool.tile([P, gsz * A], BF16, tag="h", name="h_tile")
            tanh_ins.append(
                nc.scalar.activation(out=h_tile, in_=s_tile, func=AF.Tanh))
            for j in range(gsz):
                e = e0 + j
                for ac in range(AC):
                    col = ac * P + e
                    mm = nc.tensor.matmul(
                        psum_s[mc][:, col:col + 1],
                        h_tile[:, j * A + ac * P: j * A + (ac + 1) * P],
                        w2_sb[:, mc:mc + 1],
                        start=True, stop=True)
            mm_last.append(mm)
            e0 += gsz
            it += 1

    # ---- softmax pieces -----------------------------------------------

    dve_absorb(mm_last[-1], "dve-tail-abs")
    scores_sb = const.tile([P, A], F32)
    nc.vector.tensor_copy(out=scores_sb, in_=psum_s[0])
    nc.vector.tensor_tensor(
        out=scores_sb, in0=scores_sb, in1=psum_s[1], op=ALU.add)
    expT_bf = const.tile([P, A], BF16)
    sc_exp = nc.scalar.activation(out=expT_bf, in_=scores_sb, func=AF.Exp)

    pden = ps_b.tile([P, 1], F32, tag="den")
    for ac in range(AC):
        nc.tensor.matmul(
            pden, expT_bf[:, ac * P:(ac + 1) * P], ones_cb,
            start=(ac == 0), stop=(ac == AC - 1))
    rden_sb = const.tile([P, 1], F32)
    nc.vector.reciprocal(out=rden_sb, in_=pden)

    # ---- pooledT [h, e] (unnormalized, bf16 inputs) -------------------
    poolT_sb = []
    for hc in range(HC):
        ppt = ps_b.tile([P, P], F32, tag="pT", bufs=2, name="ppt")
        for ac in range(AC):
            nc.tensor.matmul(
                ppt, wa_bf[ac][:, hc * P:(hc + 1) * P],
                expT_bf[:, ac * P:(ac + 1) * P],
                start=(ac == 0), stop=(ac == AC - 1))
        t = const.tile([P, P], F32, name=f"poolT_sb{hc}")
        nc.vector.tensor_copy(out=t, in_=ppt)
        poolT_sb.append(t)

    # ---- final: out = rden * (poolT.T @ w3) + (ww @ w3 + b3) ----------
    pq1 = ps_b.tile([P, M], F32, tag="q1")
    pq2 = ps_b.tile([P, M], F32, tag="q2")
    for hc in range(HC):
        q1_last = nc.tensor.matmul(pq1, poolT_sb[hc], w3_sb[hc],
                                   start=(hc == 0), stop=(hc == HC - 1))
        nc.tensor.matmul(pq2, wwT_sb[hc], w3_sb[hc],
                         start=(hc == 0), stop=False)
    q2_last = nc.tensor.matmul(pq2, ones_f[0:1, 0:P], b3_sb,
                               start=False, stop=True)

    dve_absorb(q1_last, "dve-q1-abs")
    t1_sb = const.tile([P, M], F32)
    nc.vector.tensor_scalar(
        out=t1_sb, in0=pq1, scalar1=rden_sb, scalar2=None, op0=ALU.mult)
    dve_absorb(q2_last, "dve-q2-abs")
    out_sb = const.tile([P, M], F32)
    out_w = nc.vector.tensor_tensor(out=out_sb, in0=t1_sb, in1=pq2,
                                    op=ALU.add)
    # Output via SWDGE: HWDGE DMAs always carry an own-lane FIFO wait, so
    # lane+data would exceed the 1-wait limit.  The SWDGE lane set has a
    # virgin lane here, leaving only the DVE data wait.
    out_dma = nc.gpsimd.dma_start(out=out_d, in_=out_sb)

    # SP nop joins: bring SP's vector clock up to date on every loose sem
    # end so the Tile kernel-tail drain needs no sync waits of its own.
    tail_deps = [out_dma, q2_last, q1_last, mm_last[-1], out_w, sc_exp,
                 pool_last, warm, ident_load]
    tail_deps += hw_loads + sw_loads
    for k, dep in enumerate(tail_deps):
        nop = nc.sync.nop(nofuse=True)
        bass_rust.add_dep_helper(
            nop.ins, dep.ins, sync=True, reason=f"sp-tail-join-{k}")


_NC_CACHE = None


def _get_nc():
    global _NC_CACHE
    if _NC_CACHE is None:
        _NC_CACHE = _build_kernel()
    return _NC_CACHE


def kernel(**inputs):
    wa = np.ascontiguousarray(np.asarray(inputs["word_all"], dtype=np.float32))
    ww = np.ascontiguousarray(
        np.asarray(inputs["word_weighted"], dtype=np.float32))
    w1 = np.ascontiguousarray(np.asarray(inputs["w1"], dtype=np.float32))
    b1 = np.ascontiguousarray(np.asarray(inputs["b1"], dtype=np.float32))
    w2 = np.ascontiguousarray(np.asarray(inputs["w2"], dtype=np.float32))
    w3 = np.ascontiguousarray(np.asarray(inputs["w3"], dtype=np.float32))
    b3 = np.ascontiguousarray(np.asarray(inputs["b3"], dtype=np.float32))
    # b2 is a pre-softmax additive constant: softmax(x + c) == softmax(x).

    nc = _get_nc()
    in_maps = [
        {
            "wa": np.ascontiguousarray(wa[b]),
            "ww": np.ascontiguousarray(ww[b]),
            "w1": w1,
            "b1": b1,
            "w2": w2,
            "w3": w3,
            "b3": b3,
        }
        for b in range(N_CORES)
    ]
    res = run_bass_kernel_spmd(nc, in_maps, core_ids=list(range(N_CORES)))
    return np.stack([res.results[b]["out"] for b in range(N_CORES)], axis=0)


# revision 36
# speedup vs baseline: 4397.0196x; 1.0041x over previous
"""Trainium2 Bass kernel for nn_DocSelfAttention.

Reference computation (per batch b):
    diff[e,a,h]  = wa[a,h] - ww[e,h]
    h3[e,a,m]    = tanh(diff @ w1 + b1)
    scores[e,a]  = h3 @ w2 + b2
    attn         = softmax(scores, axis=a)        (b2 cancels)
    pooled[e,h]  = attn @ wa
    out[e,m]     = (pooled + ww) @ w3 + b3

Key factorization: diff @ w1 = (wa @ w1)[a] - (ww @ w1)[e], so the big
[E,A,H]x[H,M] einsum collapses to two small matmuls plus a broadcast
subtract.  The kernel is then ACT-bound on the E*A*M = 16.7M-element tanh
per core (1 elem/cycle/lane @ 1.2 GHz ~= 112us).

Sharding: data-parallel over batch, one batch element per core (B=8).

Per-core dataflow (partition dim first):
    uT[m,a]    = (wa @ w1 + b1)^T     bf16
    vT[m,e]    = (ww @ w1)^T          f32 (per-partition scalar source)
    s/h tiles  [128m, G*512a]         bf16: tensor_scalar sub, ACT tanh
    scoresT    psum [128 a_loc, (ac,e)] via per-column matmuls
               (lhsT = h-slice [128m,128a], rhs = w2 chunk [128m,1])
    pooledT    psum [128h, 128e] = sum_ac wa_chunk.T @ expT_chunk
               (unnormalized; softmax denominator folded in at the end:
                out = rden (*) (pooledT.T @ w3) + (ww @ w3 + b3))

Walrus on this stack accepts at most ONE sync wait per engine
instruction, so the kernel maintains each engine's vector clock
explicitly: tiny PE "absorber" matmuls consume DMA/memset completions
phase by phase, and tiny DVE memsets into the fresh s/h tile slots take
over the slot-WAR waits that would otherwise land as a second wait on
the subs/tanh instructions.
"""

import numpy as np
from contextlib import ExitStack

import bass_rust
import concourse.bass as bass
import concourse.mybir as mybir
import concourse.tile as tile
from concourse.bass_utils import run_bass_kernel_spmd

F32 = mybir.dt.float32
BF16 = mybir.dt.bfloat16
AF = mybir.ActivationFunctionType
ALU = mybir.AluOpType

B, A, E, H, M = 8, 512, 128, 512, 256
P = 128
HC, MC, AC = H // P, M // P, A // P  # 4, 2, 4
G = 16                               # e-group size for sub/tanh tiles
NG = E // G                          # 8 groups

N_CORES = 8


def _build_kernel(ng=NG):
    nc = bass.Bass("TRN2", num_devices=N_CORES)

    wa_d = nc.dram_tensor("wa", [A, H], F32, kind="ExternalInput").ap()
    ww_d = nc.dram_tensor("ww", [E, H], F32, kind="ExternalInput").ap()
    w1_d = nc.dram_tensor("w1", [H, M], F32, kind="ExternalInput").ap()
    b1_d = nc.dram_tensor("b1", [M], F32, kind="ExternalInput").ap()
    w2_d = nc.dram_tensor("w2", [M], F32, kind="ExternalInput").ap()
    w3_d = nc.dram_tensor("w3", [H, M], F32, kind="ExternalInput").ap()
    b3_d = nc.dram_tensor("b3", [M], F32, kind="ExternalInput").ap()
    out_d = nc.dram_tensor("out", [E, M], F32, kind="ExternalOutput").ap()

    ident_d = nc.inline_tensor(np.eye(P, dtype=np.float32), name="ident").ap()

    with tile.TileContext(nc) as tc:
        with ExitStack() as ctx:
            _body(ctx, tc, nc, wa_d, ww_d, w1_d, b1_d, w2_d, w3_d, b3_d,
                  out_d, ident_d, ng)
    return nc


def _body(ctx, tc, nc, wa_d, ww_d, w1_d, b1_d, w2_d, w3_d, b3_d, out_d,
          ident_d, ng=NG):
    const = ctx.enter_context(tc.tile_pool(name="const", bufs=1))
    s_pool = ctx.enter_context(tc.tile_pool(name="s_pool", bufs=3))
    h_pool = ctx.enter_context(tc.tile_pool(name="h_pool", bufs=3))
    scr_pool = ctx.enter_context(tc.tile_pool(name="scr_pool", bufs=40))

    # ---- input DMAs ---------------------------------------------------
    hw_loads = []
    sw_loads = []

    ident = const.tile([P, P], F32)
    ident_load = nc.sync.dma_start(out=ident, in_=ident_d)

    act_warm = const.tile([1, 1], F32)
    warm = nc.scalar.activation(out=act_warm, in_=ident[0:1, 0:1],
                                func=AF.Tanh)

    wa_all = const.tile([P, AC, H], F32)
    hw_loads.append(nc.sync.dma_start(
        out=wa_all, in_=wa_d.rearrange("(c p) h -> p c h", p=P)))
    wa_sb = [wa_all[:, ac, :] for ac in range(AC)]

    ww_sb = const.tile([P, H], F32)
    hw_loads.append(nc.sync.dma_start(out=ww_sb, in_=ww_d))
    phaseA = [ident_load] + list(hw_loads)

    # keep the big wa DMA at the head of the SP DMA queue: everything on
    # the startup critical path waits for it
    wa_dma = hw_loads[0]
    bass_rust.add_dep_helper(
        hw_loads[1].ins, wa_dma.ins, sync=False, reason="dma-order-ww")

    w1_all = const.tile([P, HC, M], F32)
    _d = nc.sync.dma_start(
        out=w1_all, in_=w1_d.rearrange("(c p) m -> p c m", p=P))
    bass_rust.add_dep_helper(_d.ins, wa_dma.ins, sync=False,
                             reason="dma-order-w1")
    hw_loads.append(_d)
    w1_sb = [w1_all[:, hc, :] for hc in range(HC)]
    w1_ball = const.tile([P, HC, M], BF16)
    sw_loads.append(nc.gpsimd.dma_start(
        out=w1_ball, in_=w1_d.rearrange("(c p) m -> p c m", p=P)))
    w1_bf = [w1_ball[:, hc, :] for hc in range(HC)]
    w3_all = const.tile([P, HC, M], F32)
    _d = nc.sync.dma_start(
        out=w3_all, in_=w3_d.rearrange("(c p) m -> p c m", p=P))
    bass_rust.add_dep_helper(_d.ins, wa_dma.ins, sync=False,
                             reason="dma-order-w3")
    hw_loads.append(_d)
    w3_sb = [w3_all[:, hc, :] for hc in range(HC)]

    b1_bf = const.tile([1, M], BF16)
    sw_loads.append(nc.gpsimd.dma_start(
        out=b1_bf, in_=b1_d.rearrange("(o m) -> o m", o=1)))
    b3_sb = const.tile([1, M], F32)
    _d = nc.sync.dma_start(
        out=b3_sb, in_=b3_d.rearrange("(o m) -> o m", o=1))
    bass_rust.add_dep_helper(_d.ins, wa_dma.ins, sync=False,
                             reason="dma-order-b3")
    hw_loads.append(_d)

    # w2 as [128, 2] bf16 (cast during SWDGE DMA); column c = chunk c
    w2_sb = const.tile([P, MC], BF16)
    w2_load = nc.gpsimd.dma_start(
        out=w2_sb, in_=w2_d.rearrange("(c p) -> p c", p=P))
    sw_loads.append(w2_load)

    ones_bf = const.tile([1, A], BF16)
    m1 = nc.gpsimd.memset(ones_bf, 1.0)
    ones_f = const.tile([1, A], F32)
    m2 = nc.gpsimd.memset(ones_f, 1.0)
    ones_cb = const.tile([P, 1], BF16)
    pool_last = nc.gpsimd.memset(ones_cb, 1.0)

    phaseB = list(hw_loads[2:]) + sw_loads + [m1, m2, pool_last]

    # ---- psum phase A -------------------------------------------------
    wwT_sb = []
    waT_bf = [const.tile([P, A], BF16, name=f"waT_bf{hc}")
              for hc in range(HC)]
    wa_bf = [const.tile([P, H], BF16, name=f"wa_bf{ac}")
             for ac in range(AC)]
    uT_sb = []
    vT_sb = []
    w3_bf = []

    with tc.tile_pool(name="ps_a", bufs=1, space="PSUM") as ps_a:
        prime_ps = ps_a.tile([1, 1], F32, tag="prime", name="prime_ps")

        def absorb(dep, reason):
            mm = nc.tensor.matmul(
                prime_ps, ident[0:1, 0:1], ident[0:1, 0:1],
                start=True, stop=True)
            bass_rust.add_dep_helper(
                mm.ins, dep.ins, sync=True, reason=reason)
            return mm

        last_abs = None
        for k, ld in enumerate(phaseA):
            last_abs = absorb(ld, f"pe-primeA-{k}")

        def ordered(ins):
            bass_rust.add_dep_helper(
                ins.ins, last_abs.ins, sync=False, reason="pe-order")
            return ins

        # ---- waT (cast to bf16) / wwT (f32) via PE transpose ----------
        startup_ops = []
        last_T = None
        for hc in range(HC):
            for ac in range(AC):
                ptile = ps_a.tile([P, P], F32, tag="tww", bufs=4,
                                  name="pt_wa")
                last_T = ordered(nc.tensor.transpose(
                    out=ptile, in_=wa_sb[ac][:, hc * P:(hc + 1) * P],
                    identity=ident))
                startup_ops.append(nc.vector.tensor_copy(
                    out=waT_bf[hc][:, ac * P:(ac + 1) * P], in_=ptile))
        for hc in range(HC):
            ptile = ps_a.tile([P, P], F32, tag="tww", bufs=4, name="pt_ww")
            last_T = ordered(nc.tensor.transpose(
                out=ptile, in_=ww_sb[:, hc * P:(hc + 1) * P],
                identity=ident))
            t = const.tile([P, P], F32, name=f"wwT_sb{hc}")
            startup_ops.append(nc.vector.tensor_copy(out=t, in_=ptile))
            wwT_sb.append(t)

        # bf16 copies of wa (pooledT stationary later) and w3 (q1 rhs)
        for ac in range(AC):
            startup_ops.append(
                nc.vector.tensor_copy(out=wa_bf[ac], in_=wa_sb[ac]))
        for hc in range(HC):
            t = const.tile([P, M], BF16, name=f"w3_bf{hc}")
            startup_ops.append(nc.vector.tensor_copy(out=t, in_=w3_sb[hc]))
            w3_bf.append(t)

        # phase-B absorbers (w1/w3/b1/b3/w2/ones ready before u/v);
        # ordered AFTER the transposes so they don't stall them on the
        # PE FIFO while the weight DMAs are still in flight
        for k, ld in enumerate(phaseB):
            last_abs = absorb(ld, f"pe-primeB-{k}")
            bass_rust.add_dep_helper(
                last_abs.ins, last_T.ins, sync=False, reason="pe-orderB")

        # ---- uT = (wa @ w1 + b1)^T (bf16), vT = (ww @ w1)^T (f32) -----
        for mc in range(MC):
            pu = ps_a.tile([P, A], F32, tag="mm512", bufs=2, name="pu")
            for hc in range(HC):
                ordered(nc.tensor.matmul(
                    pu, w1_bf[hc][:, mc * P:(mc + 1) * P], waT_bf[hc],
                    start=(hc == 0), stop=False))
            ordered(nc.tensor.matmul(
                pu, b1_bf[0:1, mc * P:(mc + 1) * P], ones_bf,
                start=False, stop=True))
            ut = const.tile([P, A], BF16, name=f"uT_sb{mc}")
            startup_ops.append(nc.vector.tensor_copy(out=ut, in_=pu))
            uT_sb.append(ut)

            pv = ps_a.tile([P, P], F32, tag="v128", bufs=1, name="pv")
            for hc in range(HC):
                startup_ops.append(ordered(nc.tensor.matmul(
                    pv, w1_sb[hc][:, mc * P:(mc + 1) * P], wwT_sb[hc],
                    start=(hc == 0), stop=(hc == HC - 1))))
            vt = const.tile([P, P], F32, name=f"vT_sb{mc}")
            startup_ops.append(nc.vector.tensor_copy(out=vt, in_=pv))
            vT_sb.append(vt)

        # absorb all startup copies/matmuls so main-loop PE instructions
        # carry at most one fresh wait
        for k, op in enumerate(startup_ops):
            last_abs = absorb(op, f"pe-primeC-{k}")

    # ---- main loop ----------------------------------------------------
    ps_b = ctx.enter_context(tc.tile_pool(name="ps_b", bufs=1, space="PSUM"))

    # scoresT psum column (ac*128 + e) holds scores[e, ac*128 + p].
    # Separate banks per m-chunk; every matmul is its own accumulation
    # group (start=stop=True) so column order is unconstrained.
    psum_s = [ps_b.tile([P, A], F32, name=f"psum_s{mc}", tag=f"sc{mc}")
              for mc in range(MC)]

    def dve_absorb(dep, reason):
        t = scr_pool.tile([1, 1], F32, tag="dscr", name="dscr")
        ab = nc.vector.memset(t, 0.0)
        bass_rust.add_dep_helper(ab.ins, dep.ins, sync=True, reason=reason)
        return ab

    def act_absorb(dep, reason):
        t = scr_pool.tile([1, 1], F32, tag="ascr", name="ascr")
        ab = nc.scalar.copy(out=t, in_=nc.const_aps.tensor(0.0, (1, 1), F32))
        bass_rust.add_dep_helper(ab.ins, dep.ins, sync=True, reason=reason)
        return ab

    # Per-iteration absorbers keep every DVE/ACT instruction at <=1 sync
    # wait: the s-slot WAR (a previous tanh) is absorbed by a tiny DVE
    # memset, the h-slot WAR (previous scores matmuls) and the sub->tanh
    # data wait by two tiny ACT copies (the tanh's waits then collapse to
    # one ACT-own wait).
    NBUF = 3
    # Small leading groups shorten the path to the first tanh (the first
    # tanh must wait for its whole group's subs).
    groups0 = [4, 4, 8] + [G] * (ng - 1)
    assert sum(groups0) == E
    group_plan = [groups0] + [[G] * ng for _ in range(MC - 1)]
    tanh_ins = []
    mm_last = []
    it = 0
    for mc in range(MC):
        e0 = 0
        for gsz in group_plan[mc]:
            if it >= NBUF:
                dve_absorb(tanh_ins[it - NBUF], "dve-slot-abs")
            s_tile = s_pool.tile([P, gsz * A], BF16, tag="s", name="s_tile")
            for j in range(gsz):
                e = e0 + j
                sub = nc.vector.tensor_scalar(
                    out=s_tile[:, j * A:(j + 1) * A],
                    in0=uT_sb[mc],
                    scalar1=vT_sb[mc][:, e:e + 1],
                    scalar2=None,
                    op0=ALU.subtract)
            if it >= NBUF:
                act_absorb(mm_last[it - NBUF], "act-slot-abs")
            act_absorb(sub, "act-sub-abs")
            h_tile = h_pool.tile([P, gsz * A], BF16, tag="h", name="h_tile")
            tanh_ins.append(
                nc.scalar.activation(out=h_tile, in_=s_tile, func=AF.Tanh))
            for j in range(gsz):
                e = e0 + j
                for ac in range(AC):
                    col = ac * P + e
                    mm = nc.tensor.matmul(
                        psum_s[mc][:, col:col + 1],
                        h_tile[:, j * A + ac * P: j * A + (ac + 1) * P],
                        w2_sb[:, mc:mc + 1],
                        start=True, stop=True)
            mm_last.append(mm)
            e0 += gsz
            it += 1

    # ---- softmax pieces -----------------------------------------------

    dve_absorb(mm_last[-1], "dve-tail-abs")
    scores_sb = const.tile([P, A], F32)
    nc.vector.tensor_copy(out=scores_sb, in_=psum_s[0])
    nc.vector.tensor_tensor(
        out=scores_sb, in0=scores_sb, in1=psum_s[1], op=ALU.add)
    expT_bf = const.tile([P, A], BF16)
    sc_exp = nc.scalar.activation(out=expT_bf, in_=scores_sb, func=AF.Exp)

    pden = ps_b.tile([P, 1], F32, tag="den")
    for ac in range(AC):
        nc.tensor.matmul(
            pden, expT_bf[:, ac * P:(ac + 1) * P], ones_cb,
            start=(ac == 0), stop=(ac == AC - 1))
    rden_sb = const.tile([P, 1], F32)
    nc.vector.reciprocal(out=rden_sb, in_=pden)

    # ---- pooledT [h, e] (unnormalized, bf16 inputs) -------------------
    poolT_sb = []
    for hc in range(HC):
        ppt = ps_b.tile([P, P], F32, tag="pT", bufs=2, name="ppt")
        for ac in range(AC):
            nc.tensor.matmul(
                ppt, wa_bf[ac][:, hc * P:(hc + 1) * P],
                expT_bf[:, ac * P:(ac + 1) * P],
                start=(ac == 0), stop=(ac == AC - 1))
        t = const.tile([P, P], BF16, name=f"poolT_sb{hc}")
        nc.vector.tensor_copy(out=t, in_=ppt)
        poolT_sb.append(t)

    # ---- final: out = rden * (poolT.T @ w3) + (ww @ w3 + b3) ----------
    pq1 = ps_b.tile([P, M], F32, tag="q1")
    pq2 = ps_b.tile([P, M], F32, tag="q2")
    for hc in range(HC):
        q1_last = nc.tensor.matmul(pq1, poolT_sb[hc], w3_bf[hc],
                                   start=(hc == 0), stop=(hc == HC - 1))
        nc.tensor.matmul(pq2, wwT_sb[hc], w3_sb[hc],
                         start=(hc == 0), stop=False)
    q2_last = nc.tensor.matmul(pq2, ones_f[0:1, 0:P], b3_sb,
                               start=False, stop=True)

    dve_absorb(q1_last, "dve-q1-abs")
    t1_sb = const.tile([P, M], F32)
    nc.vector.tensor_scalar(
        out=t1_sb, in0=pq1, scalar1=rden_sb, scalar2=None, op0=ALU.mult)
    dve_absorb(q2_last, "dve-q2-abs")
    out_sb = const.tile([P, M], F32)
    out_w = nc.vector.tensor_tensor(out=out_sb, in0=t1_sb, in1=pq2,
                                    op=ALU.add)
    # Output via SWDGE: HWDGE DMAs always carry an own-lane FIFO wait, so
    # lane+data would exceed the 1-wait limit.  The SWDGE lane set has a
    # virgin lane here, leaving only the DVE data wait.
    out_dma = nc.gpsimd.dma_start(out=out_d, in_=out_sb)

    # SP nop joins: bring SP's vector clock up to date on every loose sem
    # end so the Tile kernel-tail drain needs no sync waits of its own.
    tail_deps = [out_dma, q2_last, q1_last, mm_last[-1], out_w, sc_exp,
                 pool_last, warm, ident_load]
    tail_deps += hw_loads + sw_loads
    for k, dep in enumerate(tail_deps):
        nop = nc.sync.nop(nofuse=True)
        bass_rust.add_dep_helper(
            nop.ins, dep.ins, sync=True, reason=f"sp-tail-join-{k}")


_NC_CACHE = None


def _get_nc():
    global _NC_CACHE
    if _NC_CACHE is None:
        _NC_CACHE = _build_kernel()
    return _NC_CACHE


def kernel(**inputs):
    wa = np.ascontiguousarray(np.asarray(inputs["word_all"], dtype=np.float32))
    ww = np.ascontiguousarray(
        np.asarray(inputs["word_weighted"], dtype=np.float32))
    w1 = np.ascontiguousarray(np.asarray(inputs["w1"], dtype=np.float32))
    b1 = np.ascontiguousarray(np.asarray(inputs["b1"], dtype=np.float32))
    w2 = np.ascontiguousarray(np.asarray(inputs["w2"], dtype=np.float32))
    w3 = np.ascontiguousarray(np.asarray(inputs["w3"], dtype=np.float32))
    b3 = np.ascontiguousarray(np.asarray(inputs["b3"], dtype=np.float32))
    # b2 is a pre-softmax additive constant: softmax(x + c) == softmax(x).

    nc = _get_nc()
    in_maps = [
        {
            "wa": np.ascontiguousarray(wa[b]),
            "ww": np.ascontiguousarray(ww[b]),
            "w1": w1,
            "b1": b1,
            "w2": w2,
            "w3": w3,
            "b3": b3,
        }
        for b in range(N_CORES)
    ]
    res = run_bass_kernel_spmd(nc, in_maps, core_ids=list(range(N_CORES)))
    return np.stack([res.results[b]["out"] for b in range(N_CORES)], axis=0)


# revision 37
# speedup vs baseline: 4433.3911x; 1.0083x over previous
"""Trainium2 Bass kernel for nn_DocSelfAttention.

Reference computation (per batch b):
    diff[e,a,h]  = wa[a,h] - ww[e,h]
    h3[e,a,m]    = tanh(diff @ w1 + b1)
    scores[e,a]  = h3 @ w2 + b2
    attn         = softmax(scores, axis=a)        (b2 cancels)
    pooled[e,h]  = attn @ wa
    out[e,m]     = (pooled + ww) @ w3 + b3

Key factorization: diff @ w1 = (wa @ w1)[a] - (ww @ w1)[e], so the big
[E,A,H]x[H,M] einsum collapses to two small matmuls plus a broadcast
subtract.  The kernel is then ACT-bound on the E*A*M = 16.7M-element tanh
per core (1 elem/cycle/lane @ 1.2 GHz ~= 112us).

Sharding: data-parallel over batch, one batch element per core (B=8).

Per-core dataflow (partition dim first):
    uT[m,a]    = (wa @ w1 + b1)^T     bf16
    vT[m,e]    = (ww @ w1)^T          f32 (per-partition scalar source)
    s/h tiles  [128m, G*512a]         bf16: tensor_scalar sub, ACT tanh
    scoresT    psum [128 a_loc, (ac,e)] via per-column matmuls
               (lhsT = h-slice [128m,128a], rhs = w2 chunk [128m,1])
    pooledT    psum [128h, 128e] = sum_ac wa_chunk.T @ expT_chunk
               (unnormalized; softmax denominator folded in at the end:
                out = rden (*) (pooledT.T @ w3) + (ww @ w3 + b3))

Walrus on this stack accepts at most ONE sync wait per engine
instruction, so the kernel maintains each engine's vector clock
explicitly: tiny PE "absorber" matmuls consume DMA/memset completions
phase by phase, and tiny DVE memsets into the fresh s/h tile slots take
over the slot-WAR waits that would otherwise land as a second wait on
the subs/tanh instructions.
"""

import numpy as np
from contextlib import ExitStack

import bass_rust
import concourse.bass as bass
import concourse.mybir as mybir
import concourse.tile as tile
from concourse.bass_utils import run_bass_kernel_spmd

F32 = mybir.dt.float32
BF16 = mybir.dt.bfloat16
AF = mybir.ActivationFunctionType
ALU = mybir.AluOpType

B, A, E, H, M = 8, 512, 128, 512, 256
P = 128
HC, MC, AC = H // P, M // P, A // P  # 4, 2, 4
G = 16                               # e-group size for sub/tanh tiles
NG = E // G                          # 8 groups

N_CORES = 8


def _build_kernel(ng=NG):
    nc = bass.Bass("TRN2", num_devices=N_CORES)

    wa_d = nc.dram_tensor("wa", [A, H], F32, kind="ExternalInput").ap()
    ww_d = nc.dram_tensor("ww", [E, H], F32, kind="ExternalInput").ap()
    w1_d = nc.dram_tensor("w1", [H, M], F32, kind="ExternalInput").ap()
    b1_d = nc.dram_tensor("b1", [M], F32, kind="ExternalInput").ap()
    w2_d = nc.dram_tensor("w2", [M], F32, kind="ExternalInput").ap()
    w3_d = nc.dram_tensor("w3", [H, M], F32, kind="ExternalInput").ap()
    b3_d = nc.dram_tensor("b3", [M], F32, kind="ExternalInput").ap()
    out_d = nc.dram_tensor("out", [E, M], F32, kind="ExternalOutput").ap()

    ident_d = nc.inline_tensor(np.eye(P, dtype=np.float32), name="ident").ap()

    with tile.TileContext(nc) as tc:
        with ExitStack() as ctx:
            _body(ctx, tc, nc, wa_d, ww_d, w1_d, b1_d, w2_d, w3_d, b3_d,
                  out_d, ident_d, ng)
    return nc


def _body(ctx, tc, nc, wa_d, ww_d, w1_d, b1_d, w2_d, w3_d, b3_d, out_d,
          ident_d, ng=NG):
    const = ctx.enter_context(tc.tile_pool(name="const", bufs=1))
    s_pool = ctx.enter_context(tc.tile_pool(name="s_pool", bufs=2))
    h_pool = ctx.enter_context(tc.tile_pool(name="h_pool", bufs=2))
    scr_pool = ctx.enter_context(tc.tile_pool(name="scr_pool", bufs=40))

    # ---- input DMAs ---------------------------------------------------
    hw_loads = []
    sw_loads = []

    ident = const.tile([P, P], F32)
    ident_load = nc.sync.dma_start(out=ident, in_=ident_d)

    act_warm = const.tile([1, 1], F32)
    warm = nc.scalar.activation(out=act_warm, in_=ident[0:1, 0:1],
                                func=AF.Tanh)

    wa_all = const.tile([P, AC, H], F32)
    hw_loads.append(nc.sync.dma_start(
        out=wa_all, in_=wa_d.rearrange("(c p) h -> p c h", p=P)))
    wa_sb = [wa_all[:, ac, :] for ac in range(AC)]

    ww_sb = const.tile([P, H], F32)
    hw_loads.append(nc.sync.dma_start(out=ww_sb, in_=ww_d))
    phaseA = [ident_load] + list(hw_loads)

    # keep the big wa DMA at the head of the SP DMA queue: everything on
    # the startup critical path waits for it
    wa_dma = hw_loads[0]
    bass_rust.add_dep_helper(
        hw_loads[1].ins, wa_dma.ins, sync=False, reason="dma-order-ww")

    w1_all = const.tile([P, HC, M], F32)
    _d = nc.sync.dma_start(
        out=w1_all, in_=w1_d.rearrange("(c p) m -> p c m", p=P))
    bass_rust.add_dep_helper(_d.ins, wa_dma.ins, sync=False,
                             reason="dma-order-w1")
    hw_loads.append(_d)
    w1_sb = [w1_all[:, hc, :] for hc in range(HC)]
    w1_ball = const.tile([P, HC, M], BF16)
    sw_loads.append(nc.gpsimd.dma_start(
        out=w1_ball, in_=w1_d.rearrange("(c p) m -> p c m", p=P)))
    w1_bf = [w1_ball[:, hc, :] for hc in range(HC)]
    w3_all = const.tile([P, HC, M], F32)
    _d = nc.sync.dma_start(
        out=w3_all, in_=w3_d.rearrange("(c p) m -> p c m", p=P))
    bass_rust.add_dep_helper(_d.ins, wa_dma.ins, sync=False,
                             reason="dma-order-w3")
    hw_loads.append(_d)
    w3_sb = [w3_all[:, hc, :] for hc in range(HC)]

    b1_bf = const.tile([1, M], BF16)
    sw_loads.append(nc.gpsimd.dma_start(
        out=b1_bf, in_=b1_d.rearrange("(o m) -> o m", o=1)))
    b3_sb = const.tile([1, M], F32)
    _d = nc.sync.dma_start(
        out=b3_sb, in_=b3_d.rearrange("(o m) -> o m", o=1))
    bass_rust.add_dep_helper(_d.ins, wa_dma.ins, sync=False,
                             reason="dma-order-b3")
    hw_loads.append(_d)

    # w2 as [128, 2] bf16 (cast during SWDGE DMA); column c = chunk c
    w2_sb = const.tile([P, MC], BF16)
    w2_load = nc.gpsimd.dma_start(
        out=w2_sb, in_=w2_d.rearrange("(c p) -> p c", p=P))
    sw_loads.append(w2_load)

    ones_bf = const.tile([1, A], BF16)
    m1 = nc.gpsimd.memset(ones_bf, 1.0)
    ones_f = const.tile([1, A], F32)
    m2 = nc.gpsimd.memset(ones_f, 1.0)
    ones_cb = const.tile([P, 1], BF16)
    pool_last = nc.gpsimd.memset(ones_cb, 1.0)

    phaseB = list(hw_loads[2:]) + sw_loads + [m1, m2, pool_last]

    # ---- psum phase A -------------------------------------------------
    wwT_sb = []
    waT_bf = [const.tile([P, A], BF16, name=f"waT_bf{hc}")
              for hc in range(HC)]
    wa_bf = [const.tile([P, H], BF16, name=f"wa_bf{ac}")
             for ac in range(AC)]
    uT_sb = []
    vT_sb = []
    w3_bf = []

    with tc.tile_pool(name="ps_a", bufs=1, space="PSUM") as ps_a:
        prime_ps = ps_a.tile([1, 1], F32, tag="prime", name="prime_ps")

        def absorb(dep, reason):
            mm = nc.tensor.matmul(
                prime_ps, ident[0:1, 0:1], ident[0:1, 0:1],
                start=True, stop=True)
            bass_rust.add_dep_helper(
                mm.ins, dep.ins, sync=True, reason=reason)
            return mm

        last_abs = None
        for k, ld in enumerate(phaseA):
            last_abs = absorb(ld, f"pe-primeA-{k}")

        def ordered(ins):
            bass_rust.add_dep_helper(
                ins.ins, last_abs.ins, sync=False, reason="pe-order")
            return ins

        # ---- waT (cast to bf16) / wwT (f32) via PE transpose ----------
        startup_ops = []
        last_T = None
        for hc in range(HC):
            for ac in range(AC):
                ptile = ps_a.tile([P, P], F32, tag="tww", bufs=4,
                                  name="pt_wa")
                last_T = ordered(nc.tensor.transpose(
                    out=ptile, in_=wa_sb[ac][:, hc * P:(hc + 1) * P],
                    identity=ident))
                startup_ops.append(nc.vector.tensor_copy(
                    out=waT_bf[hc][:, ac * P:(ac + 1) * P], in_=ptile))
        for hc in range(HC):
            ptile = ps_a.tile([P, P], F32, tag="tww", bufs=4, name="pt_ww")
            last_T = ordered(nc.tensor.transpose(
                out=ptile, in_=ww_sb[:, hc * P:(hc + 1) * P],
                identity=ident))
            t = const.tile([P, P], F32, name=f"wwT_sb{hc}")
            startup_ops.append(nc.vector.tensor_copy(out=t, in_=ptile))
            wwT_sb.append(t)

        # bf16 copies of wa (pooledT stationary later) and w3 (q1 rhs)
        for ac in range(AC):
            startup_ops.append(
                nc.vector.tensor_copy(out=wa_bf[ac], in_=wa_sb[ac]))
        for hc in range(HC):
            t = const.tile([P, M], BF16, name=f"w3_bf{hc}")
            startup_ops.append(nc.vector.tensor_copy(out=t, in_=w3_sb[hc]))
            w3_bf.append(t)

        # phase-B absorbers (w1/w3/b1/b3/w2/ones ready before u/v);
        # ordered AFTER the transposes so they don't stall them on the
        # PE FIFO while the weight DMAs are still in flight
        for k, ld in enumerate(phaseB):
            last_abs = absorb(ld, f"pe-primeB-{k}")
            bass_rust.add_dep_helper(
                last_abs.ins, last_T.ins, sync=False, reason="pe-orderB")

        # ---- uT = (wa @ w1 + b1)^T (bf16), vT = (ww @ w1)^T (f32) -----
        for mc in range(MC):
            pu = ps_a.tile([P, A], F32, tag="mm512", bufs=2, name="pu")
            for hc in range(HC):
                ordered(nc.tensor.matmul(
                    pu, w1_bf[hc][:, mc * P:(mc + 1) * P], waT_bf[hc],
                    start=(hc == 0), stop=False))
            ordered(nc.tensor.matmul(
                pu, b1_bf[0:1, mc * P:(mc + 1) * P], ones_bf,
                start=False, stop=True))
            ut = const.tile([P, A], BF16, name=f"uT_sb{mc}")
            startup_ops.append(nc.vector.tensor_copy(out=ut, in_=pu))
            uT_sb.append(ut)

            pv = ps_a.tile([P, P], F32, tag="v128", bufs=1, name="pv")
            for hc in range(HC):
                startup_ops.append(ordered(nc.tensor.matmul(
                    pv, w1_sb[hc][:, mc * P:(mc + 1) * P], wwT_sb[hc],
                    start=(hc == 0), stop=(hc == HC - 1))))
            vt = const.tile([P, P], F32, name=f"vT_sb{mc}")
            startup_ops.append(nc.vector.tensor_copy(out=vt, in_=pv))
            vT_sb.append(vt)

        # absorb all startup copies/matmuls so main-loop PE instructions
        # carry at most one fresh wait
        for k, op in enumerate(startup_ops):
            last_abs = absorb(op, f"pe-primeC-{k}")

    # ---- main loop ----------------------------------------------------
    ps_b = ctx.enter_context(tc.tile_pool(name="ps_b", bufs=1, space="PSUM"))

    # scoresT psum column (ac*128 + e) holds scores[e, ac*128 + p].
    # Separate banks per m-chunk; every matmul is its own accumulation
    # group (start=stop=True) so column order is unconstrained.
    psum_s = [ps_b.tile([P, A], F32, name=f"psum_s{mc}", tag=f"sc{mc}")
              for mc in range(MC)]

    def dve_absorb(dep, reason):
        t = scr_pool.tile([1, 1], F32, tag="dscr", name="dscr")
        ab = nc.vector.memset(t, 0.0)
        bass_rust.add_dep_helper(ab.ins, dep.ins, sync=True, reason=reason)
        return ab

    def act_absorb(dep, reason):
        t = scr_pool.tile([1, 1], F32, tag="ascr", name="ascr")
        ab = nc.scalar.copy(out=t, in_=nc.const_aps.tensor(0.0, (1, 1), F32))
        bass_rust.add_dep_helper(ab.ins, dep.ins, sync=True, reason=reason)
        return ab

    # Per-iteration absorbers keep every DVE/ACT instruction at <=1 sync
    # wait: the s-slot WAR (a previous tanh) is absorbed by a tiny DVE
    # memset, the h-slot WAR (previous scores matmuls) and the sub->tanh
    # data wait by two tiny ACT copies (the tanh's waits then collapse to
    # one ACT-own wait).
    NBUF = 2
    # Small leading groups shorten the path to the first tanh (the first
    # tanh must wait for its whole group's subs); later groups are large
    # to amortize the per-instruction init and absorber costs.
    group_plan = [[4, 4, 8, 16, 32, 32, 32], [32, 32, 32, 32]]
    assert all(sum(gp) == E for gp in group_plan)
    tanh_ins = []
    mm_last = []
    it = 0
    for mc in range(MC):
        e0 = 0
        for gsz in group_plan[mc]:
            if it >= NBUF:
                dve_absorb(tanh_ins[it - NBUF], "dve-slot-abs")
            s_tile = s_pool.tile([P, gsz * A], BF16, tag="s", name="s_tile")
            for j in range(gsz):
                e = e0 + j
                sub = nc.vector.tensor_scalar(
                    out=s_tile[:, j * A:(j + 1) * A],
                    in0=uT_sb[mc],
                    scalar1=vT_sb[mc][:, e:e + 1],
                    scalar2=None,
                    op0=ALU.subtract)
            if it >= NBUF:
                act_absorb(mm_last[it - NBUF], "act-slot-abs")
            act_absorb(sub, "act-sub-abs")
            h_tile = h_pool.tile([P, gsz * A], BF16, tag="h", name="h_tile")
            tanh_ins.append(
                nc.scalar.activation(out=h_tile, in_=s_tile, func=AF.Tanh))
            for j in range(gsz):
                e = e0 + j
                for ac in range(AC):
                    col = ac * P + e
                    mm = nc.tensor.matmul(
                        psum_s[mc][:, col:col + 1],
                        h_tile[:, j * A + ac * P: j * A + (ac + 1) * P],
                        w2_sb[:, mc:mc + 1],
                        start=True, stop=True)
            mm_last.append(mm)
            e0 += gsz
            it += 1

    # ---- softmax pieces -----------------------------------------------

    dve_absorb(mm_last[-1], "dve-tail-abs")
    scores_sb = const.tile([P, A], F32)
    nc.vector.tensor_copy(out=scores_sb, in_=psum_s[0])
    nc.vector.tensor_tensor(
        out=scores_sb, in0=scores_sb, in1=psum_s[1], op=ALU.add)
    expT_bf = const.tile([P, A], BF16)
    sc_exp = nc.scalar.activation(out=expT_bf, in_=scores_sb, func=AF.Exp)

    pden = ps_b.tile([P, 1], F32, tag="den")
    for ac in range(AC):
        nc.tensor.matmul(
            pden, expT_bf[:, ac * P:(ac + 1) * P], ones_cb,
            start=(ac == 0), stop=(ac == AC - 1))
    rden_sb = const.tile([P, 1], F32)
    nc.vector.reciprocal(out=rden_sb, in_=pden)

    # ---- pooledT [h, e] (unnormalized, bf16 inputs) -------------------
    poolT_sb = []
    for hc in range(HC):
        ppt = ps_b.tile([P, P], F32, tag="pT", bufs=2, name="ppt")
        for ac in range(AC):
            nc.tensor.matmul(
                ppt, wa_bf[ac][:, hc * P:(hc + 1) * P],
                expT_bf[:, ac * P:(ac + 1) * P],
                start=(ac == 0), stop=(ac == AC - 1))
        t = const.tile([P, P], BF16, name=f"poolT_sb{hc}")
        nc.vector.tensor_copy(out=t, in_=ppt)
        poolT_sb.append(t)

    # ---- final: out = rden * (poolT.T @ w3) + (ww @ w3 + b3) ----------
    pq1 = ps_b.tile([P, M], F32, tag="q1")
    pq2 = ps_b.tile([P, M], F32, tag="q2")
    for hc in range(HC):
        q1_last = nc.tensor.matmul(pq1, poolT_sb[hc], w3_bf[hc],
                                   start=(hc == 0), stop=(hc == HC - 1))
        nc.tensor.matmul(pq2, wwT_sb[hc], w3_sb[hc],
                         start=(hc == 0), stop=False)
    q2_last = nc.tensor.matmul(pq2, ones_f[0:1, 0:P], b3_sb,
                               start=False, stop=True)

    dve_absorb(q1_last, "dve-q1-abs")
    t1_sb = const.tile([P, M], F32)
    nc.vector.tensor_scalar(
        out=t1_sb, in0=pq1, scalar1=rden_sb, scalar2=None, op0=ALU.mult)
    dve_absorb(q2_last, "dve-q2-abs")
    out_sb = const.tile([P, M], F32)
    out_w = nc.vector.tensor_tensor(out=out_sb, in0=t1_sb, in1=pq2,
                                    op=ALU.add)
    # Output via SWDGE: HWDGE DMAs always carry an own-lane FIFO wait, so
    # lane+data would exceed the 1-wait limit.  The SWDGE lane set has a
    # virgin lane here, leaving only the DVE data wait.
    out_dma = nc.gpsimd.dma_start(out=out_d, in_=out_sb)

    # SP nop joins: bring SP's vector clock up to date on every loose sem
    # end so the Tile kernel-tail drain needs no sync waits of its own.
    tail_deps = [out_dma, q2_last, q1_last, mm_last[-1], out_w, sc_exp,
                 pool_last, warm, ident_load]
    tail_deps += hw_loads + sw_loads
    for k, dep in enumerate(tail_deps):
        nop = nc.sync.nop(nofuse=True)
        bass_rust.add_dep_helper(
            nop.ins, dep.ins, sync=True, reason=f"sp-tail-join-{k}")


_NC_CACHE = None


def _get_nc():
    global _NC_CACHE
    if _NC_CACHE is None:
        _NC_CACHE = _build_kernel()
    return _NC_CACHE


def kernel(**inputs):
    wa = np.ascontiguousarray(np.asarray(inputs["word_all"], dtype=np.float32))
    ww = np.ascontiguousarray(
        np.asarray(inputs["word_weighted"], dtype=np.float32))
    w1 = np.ascontiguousarray(np.asarray(inputs["w1"], dtype=np.float32))
    b1 = np.ascontiguousarray(np.asarray(inputs["b1"], dtype=np.float32))
    w2 = np.ascontiguousarray(np.asarray(inputs["w2"], dtype=np.float32))
    w3 = np.ascontiguousarray(np.asarray(inputs["w3"], dtype=np.float32))
    b3 = np.ascontiguousarray(np.asarray(inputs["b3"], dtype=np.float32))
    # b2 is a pre-softmax additive constant: softmax(x + c) == softmax(x).

    nc = _get_nc()
    in_maps = [
        {
            "wa": np.ascontiguousarray(wa[b]),
            "ww": np.ascontiguousarray(ww[b]),
            "w1": w1,
            "b1": b1,
            "w2": w2,
            "w3": w3,
            "b3": b3,
        }
        for b in range(N_CORES)
    ]
    res = run_bass_kernel_spmd(nc, in_maps, core_ids=list(range(N_CORES)))
    return np.stack([res.results[b]["out"] for b in range(N_CORES)], axis=0)
